# revision 1
# baseline (speedup 1.0000x reference)
"""Block-sparse attention kernel for TRN2 (8 NeuronCores, SPMD).

Math (from the reference nn.Module):
  x [1, 8, 512, 768] -> flatten to [S=4096, 768]
  q/k/v = x @ W{q,k,v}.T, split into H=12 heads of D=64
  block mask: query in view v attends keys [0 : P_v] where
  P_v = 1024 for v in {0,1}, (v+1)*512 for v >= 2  (always a prefix)
  out = softmax(q k^T / 8 + mask) v, merge heads, @ Wo.T + bo

Sharding: core c owns the 512 queries of view c. Every core computes the
full K/V projections (replicated; avoids collectives) and masks keys
beyond its prefix via an exp-bias of -1e5 per 512-key block.

Dataflow is fully transposed (x^T, K^T, Q^T, out^T) so every matmul has
its contraction dim on partitions and a 512-wide moving operand
(float32r at full PE rate). Softmax sums come for free from a ones
column interleaved into the V tiles (head h occupies columns h*65..+64,
column h*65+64 is 1.0), so the AV matmul's 65th output row is the
per-query sum of exp.
"""

import sys

sys.path.insert(0, "/opt/trn_rl_repo")

import numpy as np

import concourse.bass as bass
import concourse.mybir as mybir
import concourse.tile as tile
from concourse.bass_utils import run_bass_kernel_spmd

F32 = mybir.dt.float32
F32R = mybir.dt.float32r

S, DIM, H, D = 4096, 768, 12, 64
V, L = 8, 512
NC_N = 8
NM = DIM // 128          # 6 chunks of the model dim
NKB = S // 512           # 8 key blocks
SCALE = float(D) ** -0.5
NEG = -1.0e5

# allowed 512-key blocks per view (prefix length / 512)
KB_VIEW = [2, 2, 3, 4, 5, 6, 7, 8]
# core c handles half (c%2) of views PAIR_A[c] (group A, compiled 4 kb)
# and PAIR_B[c] (group B, compiled 8 kb), 256 queries each
PAIR_A = [0, 0, 1, 1, 2, 2, 3, 3]
PAIR_B = [7, 7, 6, 6, 5, 5, 4, 4]
NKB_A = 4
LG = 256  # queries per group


def legalize_multiwaits(nc):
    """This toolchain's walrus accepts at most ONE sync-wait per
    instruction; Tile's sem-assignment happily emits several. Split the
    extras into standalone EventSemaphore (wait) instructions on the same
    engine, placed immediately before the gated instruction."""
    scratch = nc.alloc_semaphore("legalize_scratch")
    fn = nc.m.functions[0]
    for bb in fn.blocks:
        insts = bb.instructions
        out = []
        changed = False
        for inst in insts:
            si = getattr(inst, "sync_info", None)
            ow = list(si.on_wait) if si is not None and si.on_wait else []
            if len(ow) > 1:
                for w in ow[:-1]:
                    ev = nc.engines[inst.engine].nop(nofuse=True)
                    raw = ev.ins
                    raw.sync_info = mybir.SyncInfo(on_wait=[w], on_update=[])
                    # pop it from wherever the builder appended it
                    tail = nc.cur_bb.bb.instructions
                    assert tail[-1].name == raw.name
                    nc.cur_bb.bb.instructions = tail[:-1]
                    out.append(raw)
                si.on_wait = [ow[-1]]
                inst.sync_info = si
                changed = True
            out.append(inst)
        if changed:
            bb.instructions = out


class _nullcm:
    def __enter__(self):
        return None

    def __exit__(self, *a):
        return False


def build_program(nkb=NKB, loop_n=1):
    nc = bass.Bass()
    xT = nc.dram_tensor("xT", [DIM, S], F32, kind="ExternalInput")
    xTq = nc.dram_tensor("xTq", [DIM, L], F32, kind="ExternalInput")
    WqT = nc.dram_tensor("WqT", [DIM, DIM], F32, kind="ExternalInput")
    WkT = nc.dram_tensor("WkT", [DIM, DIM], F32, kind="ExternalInput")
    WvT = nc.dram_tensor("WvT", [DIM, DIM], F32, kind="ExternalInput")
    WoT = nc.dram_tensor("WoT", [DIM, DIM], F32, kind="ExternalInput")
    boT = nc.dram_tensor("boT", [128, NM], F32, kind="ExternalInput")
    maskT = nc.dram_tensor("maskT", [128, NKB_A + NKB], F32, kind="ExternalInput")
    outT = nc.dram_tensor("outT", [DIM, L], F32, kind="ExternalOutput")

    def mm(out, lhsT, rhs, start, stop):
        nc.tensor.matmul(out, lhsT, rhs, start=start, stop=stop)

    with nc.allow_low_precision(reason="f32r accumulators (4-byte)"), \
         tile.TileContext(nc) as tc, \
         tc.tile_pool(name="const", bufs=1) as cpool, \
         tc.tile_pool(name="wres", bufs=1) as wres, \
         tc.tile_pool(name="wstream", bufs=6) as wstr, \
         tc.tile_pool(name="acc", bufs=1) as accp, \
         tc.tile_pool(name="xt", bufs=2) as xtp, \
         tc.tile_pool(name="ktblk", bufs=2) as ktp, \
         tc.tile_pool(name="vblk", bufs=2) as vbp, \
         tc.tile_pool(name="expp", bufs=2) as expp, \
         tc.tile_pool(name="outp", bufs=2) as outp, \
         tc.tile_pool(name="ps_proj", bufs=2, space="PSUM") as psproj, \
         tc.tile_pool(name="ps_sc", bufs=1, space="PSUM") as pssc, \
         tc.tile_pool(name="ps_av", bufs=2, space="PSUM") as psav:

        mask_sb = cpool.tile([128, NKB_A + NKB], F32, name="mask_sb")
        nc.sync.dma_start(mask_sb[:, :], maskT[:, :])
        ones_r = cpool.tile([65, 64], F32R, name="ones_r")
        nc.vector.memset(ones_r[:, :].bitcast(F32), 1.0)
        bo_sb = cpool.tile([128, NM], F32, name="bo_sb")
        nc.sync.dma_start(bo_sb[:, :], boT[:, :])

        # resident K/V weights: block cc at cols cc*DIM, rows = W*T rows
        wk_sb = wres.tile([128, NM * DIM], F32R, name="wk_sb")
        wv_sb = wres.tile([128, NM * DIM], F32R, name="wv_sb")
        for cc in range(NM):
            nc.gpsimd.dma_start(
                wk_sb[:, cc * DIM:(cc + 1) * DIM], WkT[cc * 128:(cc + 1) * 128, :]
            )
            nc.gpsimd.dma_start(
                wv_sb[:, cc * DIM:(cc + 1) * DIM], WvT[cc * 128:(cc + 1) * 128, :]
            )

        for _rep in range(loop_n):
            # ---- Q projection: Q^T[mm-block] = sum_cc WqT[cc,mm].T @ xTq[cc] ----
            # cols [g*NM*LG + mi*LG : +LG] = group g, m-chunk mi
            qt_sb = accp.tile([128, 2 * NM * LG], F32R, name="qt_sb")
            xq_sb = xtp.tile([128, NM * L], F32R, name="xt_t", tag="xt")
            for cc in range(NM):
                nc.gpsimd.dma_start(
                    xq_sb[:, cc * L:(cc + 1) * L], xTq[cc * 128:(cc + 1) * 128, :]
                )
            wq_t = []
            for cc in range(NM):
                w = wstr.tile([128, DIM], F32R, name=f"wq_{cc}", tag="wstr")
                nc.gpsimd.dma_start(w[:, :], WqT[cc * 128:(cc + 1) * 128, :])
                wq_t.append(w)
            for mi in range(NM):
                psq = psproj.tile([128, L], F32, name="psq", tag="proj")
                for cc in range(NM):
                    mm(psq[:, :], wq_t[cc][:, mi * 128:(mi + 1) * 128],
                       xq_sb[:, cc * L:(cc + 1) * L], cc == 0, cc == NM - 1)
                # xTq is [A queries 0:256 | B queries 256:512]
                for g in range(2):
                    nc.vector.tensor_copy(
                        qt_sb[:, g * NM * LG + mi * LG: g * NM * LG + (mi + 1) * LG],
                        psq[:, g * LG:(g + 1) * LG])

            # persistent accumulators
            # per group: at_acc cols [g*NM*LG + mi*LG : +LG]
            at_acc = accp.tile([128, 2 * NM * LG], F32R, name="at_acc")
            # softmax sums: head h, group g on partition (h%2)*64, tile h//2, cols g*LG
            sums_sb = [accp.tile([65, 2 * LG], F32, name=f"sums_{t}") for t in range(6)]

            # ---- key-block loop ----
            for kb in range(nkb):
                xt_b = xtp.tile([128, NM * L], F32R, name="xt_t", tag="xt")
                for cc in range(NM):
                    nc.gpsimd.dma_start(
                        xt_b[:, cc * L:(cc + 1) * L],
                        xT[cc * 128:(cc + 1) * 128, kb * 512:(kb + 1) * 512],
                    )

                # K^T block: [dims(part, by mm), 512 keys]
                kt_b = ktp.tile([128, NM * 512], F32R, name="kt_b", tag="kt")
                for mi in range(NM):
                    psk = psproj.tile([128, 512], F32, name="psk", tag="proj")
                    for cc in range(NM):
                        mm(psk[:, :],
                           wk_sb[:, cc * DIM + mi * 128: cc * DIM + (mi + 1) * 128],
                           xt_b[:, cc * L:(cc + 1) * L], cc == 0, cc == NM - 1)
                    nc.vector.tensor_copy(kt_b[:, mi * 512:(mi + 1) * 512], psk[:, :])

                # V block: 4 sub-chunks of 128 keys; head h at cols h*65..h*65+63,
                # col h*65+64 stays 1.0 from the memset (softmax-sum trick)
                v_b = vbp.tile([128, 4 * H * (D + 1)], F32R, name="v_b", tag="v")
                nc.vector.memset(v_b[:, :].bitcast(F32), 1.0)
                v_sc = [v_b[:, sc * H * (D + 1):(sc + 1) * H * (D + 1)]
                        for sc in range(4)]
                for sc in range(4):
                    v_t = v_sc[sc]
                    for half in range(2):
                        psv = psproj.tile([128, 512], F32, name="psv", tag="proj")
                        for cc in range(NM):
                            mm(psv[:, 0:384],
                               xt_b[:, cc * L + sc * 128: cc * L + (sc + 1) * 128],
                               wv_sb[:, cc * DIM + half * 384: cc * DIM + (half + 1) * 384],
                               cc == 0, cc == NM - 1)
                        dst = v_t[:, half * 6 * 65:(half + 1) * 6 * 65]
                        dst = dst.rearrange("p (h j) -> p h j", j=65)[:, :, 0:64]
                        srcp = psv[:, 0:384].rearrange("p (h j) -> p h j", j=64)
                        nc.vector.tensor_copy(dst, srcp)

                # attention for all heads against this key block; group A only
                # participates for kb < NKB_A (its compiled prefix)
                for h in range(H):
                    po = (h % 2) * 64            # partition offset of head h
                    co = (h // 2) * 512          # col offset (mm block h//2)
                    sp = (h % 2) * 64
                    groups = [1] if kb >= NKB_A else [1, 0]
                    for g in groups:
                        qcol = g * NM * LG + (h // 2) * LG
                        mcol = kb if g == 0 else NKB_A + kb
                        ps_s = pssc.tile([128, 1024], F32, name="ps_s", tag="sc")
                        for sc in range(4):
                            mm(ps_s[:, sc * LG:(sc + 1) * LG],
                               kt_b[po:po + 64, co + sc * 128: co + (sc + 1) * 128],
                               qt_sb[po:po + 64, qcol:qcol + LG], True, True)
                        exp_t = expp.tile([128, 1024], F32R, name="exp_t", tag="exp")
                        nc.scalar.activation(
                            exp_t[:, :], ps_s[:, :],
                            mybir.ActivationFunctionType.Exp,
                            bias=mask_sb[:, mcol:mcol + 1], scale=SCALE,
                        )
                        ps_o = psav.tile([D + 1, LG], F32, name="ps_o", tag="av")
                        for sc in range(4):
                            mm(ps_o[:, :], v_sc[sc][:, h * 65:(h + 1) * 65],
                               exp_t[:, sc * LG:(sc + 1) * LG], sc == 0, sc == 3)
                        acol = g * NM * LG + (h // 2) * LG
                        scol = g * LG
                        if kb == 0:
                            nc.vector.tensor_copy(at_acc[po:po + 64, acol:acol + LG],
                                                  ps_o[0:64, :])
                            nc.vector.tensor_copy(
                                sums_sb[h // 2][sp:sp + 1, scol:scol + LG],
                                ps_o[64:65, :])
                        else:
                            nc.vector.tensor_add(at_acc[po:po + 64, acol:acol + LG],
                                                 at_acc[po:po + 64, acol:acol + LG],
                                                 ps_o[0:64, :])
                            nc.vector.tensor_add(
                                sums_sb[h // 2][sp:sp + 1, scol:scol + LG],
                                sums_sb[h // 2][sp:sp + 1, scol:scol + LG],
                                ps_o[64:65, :])

            # ---- normalize + output projection ----
            recip_sb = [accp.tile([65, 2 * LG], F32R, name=f"recip_{t}")
                        for t in range(6)]
            for t in range(6):
                nc.vector.reciprocal(recip_sb[t][:, :], sums_sb[t][:, :])
            for g in range(2):
                for mmi in range(NM):
                    # broadcast 1/sum across partitions via outer product with
                    # ones; everything at partition base 0 (PE dst-partition
                    # rule), the two heads land in different PSUM banks
                    rb_ps = pssc.tile([128, 1024], F32, name="rb_ps", tag="sc")
                    for sub in range(2):
                        h = mmi * 2 + sub
                        sp = (h % 2) * 64
                        stage_r = outp.tile([1, LG], F32R, name="stage_r", tag="stg")
                        nc.vector.tensor_copy(
                            stage_r[0:1, :],
                            recip_sb[h // 2][sp:sp + 1, g * LG:(g + 1) * LG])
                        mm(rb_ps[0:64, sub * 512:sub * 512 + LG],
                           ones_r[0:1, :], stage_r[0:1, :], True, True)
                    acol = g * NM * LG + mmi * LG
                    for sub in range(2):
                        nc.vector.tensor_mul(
                            at_acc[sub * 64:(sub + 1) * 64, acol:acol + LG],
                            at_acc[sub * 64:(sub + 1) * 64, acol:acol + LG],
                            rb_ps[0:64, sub * 512:sub * 512 + LG])

            wo_t = []
            for cc in range(NM):
                w = wstr.tile([128, DIM], F32R, name=f"wo_{cc}", tag="wstr")
                nc.gpsimd.dma_start(w[:, :], WoT[cc * 128:(cc + 1) * 128, :])
                wo_t.append(w)
            for g in range(2):
                for mmi in range(NM):
                    pso = psproj.tile([128, L], F32, name="pso", tag="proj")
                    for cc in range(NM):
                        mm(pso[:, 0:LG], wo_t[cc][:, mmi * 128:(mmi + 1) * 128],
                           at_acc[:, g * NM * LG + cc * LG: g * NM * LG + (cc + 1) * LG],
                           cc == 0, cc == NM - 1)
                    out_t = outp.tile([128, LG], F32, name="out_t", tag="out")
                    nc.vector.tensor_scalar_add(out_t[:, :], pso[:, 0:LG],
                                                bo_sb[:, mmi:mmi + 1])
                    nc.sync.dma_start(
                        outT[mmi * 128:(mmi + 1) * 128, g * LG:(g + 1) * LG],
                        out_t[:, :])

    legalize_multiwaits(nc)
    return nc


_program = None


def make_in_maps(x, Wq, Wk, Wv, Wo, bo):
    xf = np.ascontiguousarray(np.asarray(x, np.float32).reshape(S, DIM))
    xT = np.ascontiguousarray(xf.T)
    shared = {
        "xT": xT,
        "WqT": np.ascontiguousarray(np.asarray(Wq, np.float32).T),
        "WkT": np.ascontiguousarray(np.asarray(Wk, np.float32).T),
        "WvT": np.ascontiguousarray(np.asarray(Wv, np.float32).T),
        "WoT": np.ascontiguousarray(np.asarray(Wo, np.float32).T),
        "boT": np.ascontiguousarray(
            np.asarray(bo, np.float32).reshape(NM, 128).T
        ),
    }
    in_maps = []
    for c in range(NC_N):
        hf = c % 2
        va, vb = PAIR_A[c], PAIR_B[c]
        ra = va * 512 + hf * LG
        rb = vb * 512 + hf * LG
        mask = np.zeros((128, NKB_A + NKB), np.float32)
        mask[:, KB_VIEW[va]:NKB_A] = NEG
        mask[:, NKB_A + KB_VIEW[vb]:] = NEG
        m = dict(shared)
        m["xTq"] = np.ascontiguousarray(
            np.concatenate([xT[:, ra:ra + LG], xT[:, rb:rb + LG]], axis=1))
        m["maskT"] = mask
        in_maps.append(m)
    return in_maps


def kernel(x, Wq, Wk, Wv, Wo, bo):
    global _program
    in_maps = make_in_maps(x, Wq, Wk, Wv, Wo, bo)
    if _program is None:
        _program = build_program()
    ret = run_bass_kernel_spmd(_program, in_maps, list(range(NC_N)))
    out = np.empty((S, DIM), np.float32)
    for c in range(NC_N):
        hf = c % 2
        oT = ret.results[c]["outT"]
        ra = PAIR_A[c] * 512 + hf * LG
        rb = PAIR_B[c] * 512 + hf * LG
        out[ra:ra + LG, :] = oT[:, 0:LG].T
        out[rb:rb + LG, :] = oT[:, LG:2 * LG].T
    return out.reshape(1, V, L, DIM)

